# revision 15
# baseline (speedup 1.0000x reference)
"""DRConv (dynamic region-aware conv) Trainium2 kernel.

Math (per batch b, all on device):
  x_se  = 0.25*sigmoid(routing_w @ mean_hw(x) + routing_b)           # [G*T]
  Z_t   = conv3x3(x, template_t)       for t in 0..T-1               # [O, H, W]
  U     = [x_se.T | 1] contracted with exp(Alpha) over g             # [T+1, P]
  out   = sum_t Z_t * (U_t / U_T)  + bias                            # [O, H, W]
which equals the reference because the template blend commutes through
the conv; the G-sum and T-sum exchange with the K-contraction.

Sharding: data-parallel over batch B=8, one batch element per NeuronCore.
Templates/routing weights replicated. No collectives.

Device layout (per core):
  pixels live in a 58x57 plane (one shared pad column), host-prepadded
  so the kernel DMAs the plane directly (no on-device memset/copy);
  pf = (y+1)*57 + x for image pixel (y, x); plane ships with one front
  guard column (image at offset 1) for the ij=0 shift.
  conv = 9 shifted matmuls accumulating in PSUM per 128-px tile:
    Z[px, (t,o)] += x[c, 1+base+px+delta(i,j)].T @ tmpl[c, (t,o)]
  per-pixel softmax mixing = per-partition scalar_tensor_tensor on DVE
  with pre-normalized coefficients U_t/U_T; last stt writes bf16 output
  [px, O] which the host transposes to [O, H, W].

Schedule (from trace analysis):
  - two HWDGE rings (Sync + Scalar) issue DMAs in parallel; template
    chunks alternate rings in conv consumption order
  - tiles 0-2 accumulate ij-outer-interleaved so each arriving template
    chunk feeds 6 matmuls (PE ~85% duty during the input stream)
  - tiles 3-24 run h-major (9 MMs per 512-col half) so the h0 bank
    frees at half-tile and the final tile's mix overlaps its h1 half
  - routing FC + 25 batched U-matmuls sit in the PE FIFO right after
    the interleaved block, so the PE never idles >3.4us (HAM-safe)
"""

import ml_dtypes
import numpy as np

import concourse.bass as bass
import concourse.mybir as mybir
from concourse import bacc
from concourse.tile import TileContext
from concourse.bass_utils import run_bass_kernel_spmd

# problem constants
C = 128          # in channels
O = 128          # out channels
H = W = 56
G = 8            # groups
T = 8            # num weight templates
WP = 57          # padded row width (one shared pad column)
HPAD = 58        # one pad row top and bottom
NPIX = HPAD * WP  # 3306
PLANE = 3328     # 1 front guard col + 3306 + tail guard, rounded up
PT0 = 57         # first pixel-tile starts at padded row 1
NT = 25          # 25 tiles of 128 px cover pf [57, 3257) > last valid 3247
XSPL = 1664      # x plane DMA split point
NFI = 3          # leading tiles accumulated ij-outer during the stream
NCORES = 8

_cache = {}


def _delta(ij):
    i, j = divmod(ij, 3)
    return (i - 1) * WP + (j - 1)


def _build(use_alpha: int, bias_zero: int, rb_zero: int):
    f32 = mybir.dt.float32
    bf16 = mybir.dt.bfloat16

    nc = bacc.Bacc("TRN2", target_bir_lowering=False, debug=False,
                   num_devices=NCORES)

    xp_d = nc.dram_tensor("xp", [C, PLANE], bf16, kind="ExternalInput")
    tmpl_d = nc.dram_tensor("tmpl", [9, C, T * O], bf16, kind="ExternalInput")
    rwt_d = nc.dram_tensor("rwt", [C, G * T], f32, kind="ExternalInput")
    rb_d = None
    if not rb_zero:
        rb_d = nc.dram_tensor("rb", [G * T], f32, kind="ExternalInput")
    if use_alpha:
        alpha_d = nc.dram_tensor("alpha", [G, PLANE], f32,
                                 kind="ExternalInput")
    else:
        ea_d = nc.dram_tensor("eain", [G, PLANE], bf16, kind="ExternalInput")
    if not bias_zero:
        bias_d = nc.dram_tensor("bias", [O], f32, kind="ExternalInput")
    out_d = nc.dram_tensor("out", [NT * 128, O], bf16, kind="ExternalOutput")

    with TileContext(nc) as tc:
        with (
            tc.tile_pool(name="big", bufs=1) as big,
            tc.tile_pool(name="accp", bufs=3) as accp,
            tc.tile_pool(name="ps", bufs=1, space="PSUM") as ps,
        ):
            # ---- warmup: bf16 dummies runnable immediately ----
            dummy = big.tile([128, 512], bf16)
            nc.vector.memset(dummy[:], 0.0)
            warm = ps.tile([128, 512], f32, tag="zp", bufs=7, name="warm")
            for _ in range(12):
                nc.tensor.matmul(warm[:], lhsT=dummy[:, 0:128], rhs=dummy[:])

            # ---- DMA issue plan (arrival matched to ij consumption) ----
            # sync ring:   xpA1, t0, t2, xpA2, t4, t6, t8, rwt, (rb)
            # scalar ring: alpha/ea, t1, xpB, t3, t5, t7, (out chunks)
            XA1 = 512  # covers every read of tiles 0..NFI-1
            xp = big.tile([C, PLANE], bf16)
            nc.sync.dma_start(out=xp[:, 0:XA1], in_=xp_d[:, 0:XA1])

            ea = big.tile([G, PLANE], bf16)
            if use_alpha:
                asb = big.tile([G, PLANE], f32)
                nc.scalar.dma_start(out=asb[:], in_=alpha_d[:])
            else:
                nc.scalar.dma_start(out=ea[:], in_=ea_d[:])

            tbf = []
            for ij in range(9):
                tb = big.tile([C, T * O], bf16, name=f"tbf{ij}")
                tbf.append(tb)
            nc.sync.dma_start(out=tbf[0][:], in_=tmpl_d[0])
            nc.scalar.dma_start(out=tbf[1][:], in_=tmpl_d[1])
            nc.sync.dma_start(out=tbf[2][:], in_=tmpl_d[2])
            nc.scalar.dma_start(out=xp[:, XSPL:PLANE],
                                in_=xp_d[:, XSPL:PLANE])
            nc.sync.dma_start(out=xp[:, XA1:XSPL], in_=xp_d[:, XA1:XSPL])
            nc.scalar.dma_start(out=tbf[3][:], in_=tmpl_d[3])
            nc.sync.dma_start(out=tbf[4][:], in_=tmpl_d[4])
            nc.scalar.dma_start(out=tbf[5][:], in_=tmpl_d[5])
            nc.sync.dma_start(out=tbf[6][:], in_=tmpl_d[6])
            nc.scalar.dma_start(out=tbf[7][:], in_=tmpl_d[7])
            nc.sync.dma_start(out=tbf[8][:], in_=tmpl_d[8])

            rwt = big.tile([C, G * T], f32)
            nc.sync.dma_start(out=rwt[:], in_=rwt_d[:])
            if not rb_zero:
                rb = big.tile([G * T, 1], f32)
                nc.sync.dma_start(out=rb[:], in_=rb_d[:])
            if not bias_zero:
                bias_rep = big.tile([128, O], f32)
                nc.sync.dma_start(
                    out=bias_rep[:],
                    in_=bass.AP(tensor=bias_d, offset=0,
                                ap=[[0, 128], [1, O]]),
                )

            # ---- routing probability numerators ----
            if use_alpha:
                nc.scalar.activation(ea[:], asb[:],
                                     mybir.ActivationFunctionType.Exp)

            # ---- routing GAP: split reduce so part A starts early ----
            xsA = big.tile([C, 1], f32)
            nc.vector.tensor_reduce(
                out=xsA[:], in_=xp[:, 0:XSPL],
                axis=mybir.AxisListType.X, op=mybir.AluOpType.add)
            xsum = big.tile([C, 1], f32)
            nc.vector.tensor_reduce(
                out=xsum[:], in_=xp[:, XSPL:PLANE],
                axis=mybir.AxisListType.X, op=mybir.AluOpType.add)
            nc.vector.tensor_add(xsum[:], xsum[:], xsA[:])

            # lhsT_U [g, T+1]: cols 0..T-1 = x_se[g, t], col T = 1.0
            lhsu = big.tile([G, T + 1], bf16)
            nc.vector.memset(lhsu[:, T:T + 1], 1.0)

            # U accumulator bank (in the zp rotation; freed after the
            # SBUF copy): 25 groups of 9 cols + FC columns at 228+
            upp = ps.tile([128, 240], f32, tag="zp", bufs=7, name="upp")

            # ---- output staging ----
            outbuf = big.tile([128, NT * O], bf16)

            # ---- leading tiles, ij-outer so each template chunk feeds
            # 2*NFI matmuls while the input stream is still arriving ----
            zps = {k: [ps.tile([128, 512], f32, tag="zp", bufs=7,
                               name=f"zp{h}_{k}") for h in range(2)]
                   for k in range(NFI)}
            for ij in range(9):
                for k in range(NFI):
                    base = PT0 + 128 * k
                    lo = 1 + base + _delta(ij)
                    for h in range(2):
                        nc.tensor.matmul(
                            zps[k][h][:],
                            lhsT=xp[:, lo:lo + 128],
                            rhs=tbf[ij][:, h * 512:(h + 1) * 512],
                            start=(ij == 0), stop=(ij == 8))
                if ij == 5:
                    if rb_zero:
                        # routing FC split by t so output partitions are
                        # g: upp[g, 228+t] = rwt[:, (g,t)].T @ xsum
                        rwv = rwt[:].rearrange("c (g t) -> c g t", t=T)
                        for t in range(T):
                            nc.tensor.matmul(upp[0:G, 228 + t:229 + t],
                                             lhsT=rwv[:, :, t], rhs=xsum[:])
                    else:
                        nc.tensor.matmul(upp[0:G * T, 228:229],
                                         lhsT=rwt[:], rhs=xsum[:])

            # x_se = (2/T)*sigmoid(fc(sum)/HW + rb)
            if rb_zero:
                xse8 = big.tile([G, T], f32)
                nc.scalar.activation(xse8[:], upp[0:G, 228:228 + T],
                                     mybir.ActivationFunctionType.Sigmoid,
                                     scale=1.0 / (H * W))
                nc.vector.tensor_scalar_mul(lhsu[:, 0:T], xse8[:], 2.0 / T)
            else:
                xse = big.tile([G * T, 1], f32)
                nc.scalar.activation(xse[:], upp[0:G * T, 228:229],
                                     mybir.ActivationFunctionType.Sigmoid,
                                     bias=rb[:], scale=1.0 / (H * W))
                xse4 = big.tile([G * T, 1], bf16)
                nc.vector.tensor_scalar_mul(xse4[:], xse[:], 2.0 / T)
                nc.sync.dma_start(out=lhsu[:, 0:T], in_=xse4[:])

            # batched U matmuls: up[px, (k,t)] for all 25 tiles
            for k in range(NT):
                base = PT0 + 128 * k
                nc.tensor.matmul(upp[:, 9 * k:9 * k + 9],
                                 lhsT=ea[:, base:base + 128], rhs=lhsu[:])

            # normalize once: usb[:, (k,t)] = U_t / U_T per tile
            rall = big.tile([128, NT], f32)
            upv = upp[:, 0:225].rearrange("p (k t) -> p k t", t=9)
            nc.vector.reciprocal(rall[:], upv[:, :, 8])
            usb = big.tile([128, 225], f32)
            nc.vector.tensor_tensor(
                usb[:].rearrange("p (k t) -> p k t", t=9), upv,
                rall[:, :, None].broadcast_to([128, NT, 9]),
                mybir.AluOpType.mult)

            def mix_tile(k, zp):
                acc = accp.tile([128, O], f32, tag="acc")
                for t in range(T):
                    h, tq = divmod(t, 4)
                    dst = acc[:] if t < T - 1 else outbuf[:, k * O:(k + 1) * O]
                    if t == 0:
                        if bias_zero:
                            nc.vector.tensor_scalar_mul(
                                dst, zp[0][:, 0:128], usb[:, 9 * k:9 * k + 1])
                        else:
                            nc.vector.scalar_tensor_tensor(
                                out=dst, in0=zp[0][:, 0:128],
                                scalar=usb[:, 9 * k:9 * k + 1],
                                in1=bias_rep[:],
                                op0=mybir.AluOpType.mult,
                                op1=mybir.AluOpType.add)
                    else:
                        nc.vector.scalar_tensor_tensor(
                            out=dst,
                            in0=zp[h][:, tq * 128:(tq + 1) * 128],
                            scalar=usb[:, t + 9 * k:t + 9 * k + 1],
                            in1=acc[:],
                            op0=mybir.AluOpType.mult,
                            op1=mybir.AluOpType.add)

            def store_chunk(done):
                # chunked output stores: tiles [6n, 6n+6) per DMA
                for n in range(5):
                    if done == min(6 * n + 6, NT):
                        r0 = 6 * n * 128
                        nn = done - 6 * n
                        src = outbuf[:, 6 * n * O:done * O].rearrange(
                            "p (k o) -> p k o", o=O)
                        dst = out_d[r0:r0 + nn * 128, :].rearrange(
                            "(k p) o -> p k o", p=128)
                        nc.scalar.dma_start(out=dst, in_=src)

            for k in range(NFI):
                mix_tile(k, zps.pop(k))
                store_chunk(k + 1)

            # ---- steady state: h-major conv + immediate mix ----
            for k in range(NFI, NT):
                base = PT0 + 128 * k
                zp = [ps.tile([128, 512], f32, tag="zp", bufs=7,
                              name=f"zp{h}_{k}") for h in range(2)]
                for h in range(2):
                    for ij in range(9):
                        lo = 1 + base + _delta(ij)
                        nc.tensor.matmul(
                            zp[h][:],
                            lhsT=xp[:, lo:lo + 128],
                            rhs=tbf[ij][:, h * 512:(h + 1) * 512],
                            start=(ij == 0), stop=(ij == 8))
                mix_tile(k, zp)
                store_chunk(k + 1)

    nc.compile()
    return nc


def _get(use_alpha: int, bias_zero: int, rb_zero: int):
    key = (use_alpha, bias_zero, rb_zero)
    if key not in _cache:
        _cache[key] = _build(use_alpha, bias_zero, rb_zero)
    return _cache[key]


def _in_maps(inp):
    ua = int(np.asarray(inp["use_alpha"]))
    bz = int(not np.asarray(inp["bias"]).any())
    rz = int(not np.asarray(inp["routing_b"]).any())
    x = np.asarray(inp["inputs"], dtype=np.float32).reshape(
        NCORES, C, H, W).astype(ml_dtypes.bfloat16)
    # host-prepadded plane: image row y at pf rows 1..56, cols 0..55,
    # shifted right by 1 guard col
    xp = np.zeros((NCORES, C, PLANE), dtype=ml_dtypes.bfloat16)
    v = xp[:, :, 1:1 + NPIX].reshape(NCORES, C, HPAD, WP)
    v[:, :, 1:57, 0:W] = x
    # [O*C*3*3, T] -> [(i,j), c, t*O + o]
    tmpl = np.asarray(inp["weight_templates"], dtype=np.float32).reshape(
        O, C, 3, 3, T).transpose(2, 3, 1, 4, 0).reshape(9, C, T * O)
    tmpl = np.ascontiguousarray(tmpl).astype(ml_dtypes.bfloat16)
    rwt = np.ascontiguousarray(
        np.asarray(inp["routing_w"], dtype=np.float32).T)
    rb = np.ascontiguousarray(np.asarray(inp["routing_b"], dtype=np.float32))

    if ua:
        ap = np.zeros((NCORES, G, PLANE), dtype=np.float32)
        av = ap[:, :, 1:1 + NPIX].reshape(NCORES, G, HPAD, WP)
        av[:, :, 1:57, 0:W] = np.asarray(inp["Alpha"], dtype=np.float32)
    else:
        # hard routing: one-hot(mask) in plane layout; pads -> group 0
        m = np.asarray(inp["mask"]).astype(np.int64)
        ep = np.zeros((NCORES, G, PLANE), dtype=ml_dtypes.bfloat16)
        ep[:, 0, :] = 1.0
        ev = ep[:, :, 1:1 + NPIX].reshape(NCORES, G, HPAD, WP)
        oh = (m[:, None, :, :] == np.arange(G)[None, :, None, None])
        ev[:, :, 1:57, 0:W] = oh.astype(ml_dtypes.bfloat16)

    in_maps = []
    for b in range(NCORES):
        m = {"xp": xp[b], "tmpl": tmpl, "rwt": rwt}
        if not rz:
            m["rb"] = rb
        if ua:
            m["alpha"] = ap[b]
        else:
            m["eain"] = ep[b]
        if not bz:
            m["bias"] = np.ascontiguousarray(
                np.asarray(inp["bias"], dtype=np.float32))
        in_maps.append(m)
    return in_maps, ua, bz, rz


_ROWS = (np.arange(H)[:, None] * WP + np.arange(W)[None, :]).ravel()


def kernel(inputs, mask, Alpha, weight_templates, routing_w, routing_b, bias,
           use_alpha):
    in_maps, ua, bz, rz = _in_maps(dict(
        inputs=inputs, mask=mask, Alpha=Alpha,
        weight_templates=weight_templates, routing_w=routing_w,
        routing_b=routing_b, bias=bias, use_alpha=use_alpha))
    nc = _get(ua, bz, rz)
    res = run_bass_kernel_spmd(nc, in_maps, list(range(NCORES)))
    out = np.stack([res.results[b]["out"] for b in range(NCORES)], axis=0)
    # out rows are pf-57; gather valid pixels, transpose to [O, H, W]
    out = np.asarray(out, dtype=np.float32)[:, _ROWS, :]
    out = out.transpose(0, 2, 1).reshape(NCORES, O, H, W)
    return np.ascontiguousarray(out)


# revision 19
# speedup vs baseline: 1.0342x; 1.0342x over previous
"""DRConv (dynamic region-aware conv) Trainium2 kernel.

Math (per batch b, all on device):
  x_se  = 0.25*sigmoid(routing_w @ mean_hw(x) + routing_b)           # [G*T]
  Z_t   = conv3x3(x, template_t)       for t in 0..T-1               # [O, H, W]
  U     = [x_se.T | 1] contracted with exp(Alpha) over g             # [T+1, P]
  out   = sum_t Z_t * (U_t / U_T)  + bias                            # [O, H, W]
which equals the reference because the template blend commutes through
the conv; the G-sum and T-sum exchange with the K-contraction.

Sharding: data-parallel over batch B=8, one batch element per NeuronCore.
Templates/routing weights replicated. No collectives.

Device layout (per core):
  pixels live in a 58x57 plane (one shared pad column), host-prepadded
  so the kernel DMAs the plane directly (no on-device memset/copy);
  pf = (y+1)*57 + x for image pixel (y, x); plane ships with one front
  guard column (image at offset 1) for the ij=0 shift.
  conv = 9 shifted matmuls accumulating in PSUM per 128-px tile:
    Z[px, (t,o)] += x[c, 1+base+px+delta(i,j)].T @ tmpl[c, (t,o)]
  per-pixel softmax mixing = per-partition scalar_tensor_tensor on DVE
  with pre-normalized coefficients U_t/U_T; last stt writes bf16 output
  [px, O] which the host transposes to [O, H, W].

Schedule (from trace analysis):
  - two HWDGE rings (Sync + Scalar) issue DMAs in parallel; template
    chunks alternate rings in conv consumption order
  - tiles 0-2 accumulate ij-outer-interleaved so each arriving template
    chunk feeds 6 matmuls (PE ~85% duty during the input stream)
  - tiles 3-24 run h-major (9 MMs per 512-col half) so the h0 bank
    frees at half-tile and the final tile's mix overlaps its h1 half
  - routing FC + 25 batched U-matmuls sit in the PE FIFO right after
    the interleaved block, so the PE never idles >3.4us (HAM-safe)
"""

import ml_dtypes
import numpy as np

import concourse.bass as bass
import concourse.mybir as mybir
from concourse import bacc
from concourse.tile import TileContext
from concourse.bass_utils import run_bass_kernel_spmd

# problem constants
C = 128          # in channels
O = 128          # out channels
H = W = 56
G = 8            # groups
T = 8            # num weight templates
WP = 57          # padded row width (one shared pad column)
HPAD = 58        # one pad row top and bottom
NPIX = HPAD * WP  # 3306
PLANE = 3328     # 1 front guard col + 3306 + tail guard, rounded up
PT0 = 57         # first pixel-tile starts at padded row 1
NT = 25          # 25 tiles of 128 px cover pf [57, 3257) > last valid 3247
XSPL = 1664      # x plane DMA split point
NFI = 3          # leading tiles accumulated ij-outer during the stream
NCORES = 8

_cache = {}


def _delta(ij):
    i, j = divmod(ij, 3)
    return (i - 1) * WP + (j - 1)


def _build(use_alpha: int, bias_zero: int, rb_zero: int):
    f32 = mybir.dt.float32
    bf16 = mybir.dt.bfloat16

    nc = bacc.Bacc("TRN2", target_bir_lowering=False, debug=False,
                   num_devices=NCORES)

    xp_d = nc.dram_tensor("xp", [C, PLANE], bf16, kind="ExternalInput")
    tmpl_d = nc.dram_tensor("tmpl", [C, 9 * T * O], bf16,
                            kind="ExternalInput")
    rwt_d = nc.dram_tensor("rwt", [C, G * T], f32, kind="ExternalInput")
    rb_d = None
    if not rb_zero:
        rb_d = nc.dram_tensor("rb", [G * T], f32, kind="ExternalInput")
    if use_alpha:
        alpha_d = nc.dram_tensor("alpha", [G, PLANE], f32,
                                 kind="ExternalInput")
    else:
        ea_d = nc.dram_tensor("eain", [G, PLANE], bf16, kind="ExternalInput")
    if not bias_zero:
        bias_d = nc.dram_tensor("bias", [O], f32, kind="ExternalInput")
    out_d = nc.dram_tensor("out", [NT * 128, O], bf16, kind="ExternalOutput")

    with TileContext(nc) as tc:
        with (
            tc.tile_pool(name="big", bufs=1) as big,
            tc.tile_pool(name="accp", bufs=3) as accp,
            tc.tile_pool(name="ps", bufs=1, space="PSUM") as ps,
        ):
            # ---- warmup: bf16 dummies runnable immediately ----
            dummy = big.tile([128, 512], bf16)
            nc.vector.memset(dummy[:], 0.0)
            warm = ps.tile([128, 512], f32, tag="zp", bufs=7, name="warm")
            for _ in range(6):
                nc.tensor.matmul(warm[:], lhsT=dummy[:, 0:128], rhs=dummy[:])

            # ---- DMA issue plan ----
            # <=9 input DMAs total: the per-core DMA completion-semaphore
            # pool is ~10 deep and is shared by both HWDGE rings; more
            # DMAs than that chains later issues on earlier *completions*
            # sync ring:   xpA1, tmpl chunks ij0-1,2-3,4-5,6-7,8, rwt
            # scalar ring: alpha/ea, xpRest, (rb), (out chunks)
            XA1 = 512  # covers every read of tiles 0..NFI-1
            xp = big.tile([C, PLANE], bf16)
            nc.sync.dma_start(out=xp[:, 0:XA1], in_=xp_d[:, 0:XA1])

            ea = big.tile([G, PLANE], bf16)
            if use_alpha:
                asb = big.tile([G, PLANE], f32)
                nc.scalar.dma_start(out=asb[:], in_=alpha_d[:])
            else:
                nc.scalar.dma_start(out=ea[:], in_=ea_d[:])

            tbf = big.tile([C, 9 * T * O], bf16)
            TCH = [0, 2, 4, 6, 8, 9]
            nc.sync.dma_start(out=tbf[:, 0:2048], in_=tmpl_d[:, 0:2048])
            nc.scalar.dma_start(out=xp[:, XA1:PLANE],
                                in_=xp_d[:, XA1:PLANE])
            for c in range(1, 5):
                lo, hi = TCH[c] * 1024, TCH[c + 1] * 1024
                nc.sync.dma_start(out=tbf[:, lo:hi], in_=tmpl_d[:, lo:hi])

            rwt = big.tile([C, G * T], f32)
            nc.sync.dma_start(out=rwt[:], in_=rwt_d[:])
            if not rb_zero:
                rb = big.tile([G * T, 1], f32)
                nc.scalar.dma_start(out=rb[:], in_=rb_d[:])
            if not bias_zero:
                bias_rep = big.tile([128, O], f32)
                nc.sync.dma_start(
                    out=bias_rep[:],
                    in_=bass.AP(tensor=bias_d, offset=0,
                                ap=[[0, 128], [1, O]]),
                )

            # ---- routing probability numerators ----
            if use_alpha:
                nc.scalar.activation(ea[:], asb[:],
                                     mybir.ActivationFunctionType.Exp)

            # ---- routing GAP: split reduce so part A starts early ----
            xsA = big.tile([C, 1], f32)
            nc.vector.tensor_reduce(
                out=xsA[:], in_=xp[:, 0:XA1],
                axis=mybir.AxisListType.X, op=mybir.AluOpType.add)
            xsum = big.tile([C, 1], f32)
            nc.vector.tensor_reduce(
                out=xsum[:], in_=xp[:, XA1:PLANE],
                axis=mybir.AxisListType.X, op=mybir.AluOpType.add)
            nc.vector.tensor_add(xsum[:], xsum[:], xsA[:])

            # lhsT_U [g, T+1]: cols 0..T-1 = x_se[g, t], col T = 1.0
            lhsu = big.tile([G, T + 1], bf16)
            nc.vector.memset(lhsu[:, T:T + 1], 1.0)

            # U accumulator bank (in the zp rotation; freed after the
            # SBUF copy): 25 groups of 9 cols + FC columns at 228+
            upp = ps.tile([128, 240], f32, tag="zp", bufs=7, name="upp")

            # ---- output staging ----
            outbuf = big.tile([128, NT * O], bf16)

            # ---- leading tiles, ij-outer so each template chunk feeds
            # 2*NFI matmuls while the input stream is still arriving ----
            zps = {k: [ps.tile([128, 512], f32, tag="zp", bufs=7,
                               name=f"zp{h}_{k}") for h in range(2)]
                   for k in range(NFI)}
            for ij in range(9):
                for k in range(NFI):
                    base = PT0 + 128 * k
                    lo = 1 + base + _delta(ij)
                    for h in range(2):
                        nc.tensor.matmul(
                            zps[k][h][:],
                            lhsT=xp[:, lo:lo + 128],
                            rhs=tbf[:, ij * 1024 + h * 512:
                                    ij * 1024 + (h + 1) * 512],
                            start=(ij == 0), stop=(ij == 8))
                if ij == 5:
                    if rb_zero:
                        # routing FC split by t so output partitions are
                        # g: upp[g, 228+t] = rwt[:, (g,t)].T @ xsum
                        rwv = rwt[:].rearrange("c (g t) -> c g t", t=T)
                        for t in range(T):
                            nc.tensor.matmul(upp[0:G, 228 + t:229 + t],
                                             lhsT=rwv[:, :, t], rhs=xsum[:])
                    else:
                        nc.tensor.matmul(upp[0:G * T, 228:229],
                                         lhsT=rwt[:], rhs=xsum[:])

            # x_se = (2/T)*sigmoid(fc(sum)/HW + rb)
            if rb_zero:
                xse8 = big.tile([G, T], f32)
                nc.scalar.activation(xse8[:], upp[0:G, 228:228 + T],
                                     mybir.ActivationFunctionType.Sigmoid,
                                     scale=1.0 / (H * W))
                nc.vector.tensor_scalar_mul(lhsu[:, 0:T], xse8[:], 2.0 / T)
            else:
                xse = big.tile([G * T, 1], f32)
                nc.scalar.activation(xse[:], upp[0:G * T, 228:229],
                                     mybir.ActivationFunctionType.Sigmoid,
                                     bias=rb[:], scale=1.0 / (H * W))
                xse4 = big.tile([G * T, 1], bf16)
                nc.vector.tensor_scalar_mul(xse4[:], xse[:], 2.0 / T)
                nc.sync.dma_start(out=lhsu[:, 0:T], in_=xse4[:])

            # batched U matmuls: up[px, (k,t)] for all 25 tiles
            for k in range(NT):
                base = PT0 + 128 * k
                nc.tensor.matmul(upp[:, 9 * k:9 * k + 9],
                                 lhsT=ea[:, base:base + 128], rhs=lhsu[:])

            # normalize once: usb[:, (k,t)] = U_t / U_T per tile
            rall = big.tile([128, NT], f32)
            upv = upp[:, 0:225].rearrange("p (k t) -> p k t", t=9)
            nc.vector.reciprocal(rall[:], upv[:, :, 8])
            usb = big.tile([128, 225], f32)
            nc.vector.tensor_tensor(
                usb[:].rearrange("p (k t) -> p k t", t=9), upv,
                rall[:, :, None].broadcast_to([128, NT, 9]),
                mybir.AluOpType.mult)

            def mix_tile(k, zp):
                acc = accp.tile([128, O], f32, tag="acc")
                for t in range(T):
                    h, tq = divmod(t, 4)
                    dst = acc[:] if t < T - 1 else outbuf[:, k * O:(k + 1) * O]
                    if t == 0:
                        if bias_zero:
                            nc.vector.tensor_scalar_mul(
                                dst, zp[0][:, 0:128], usb[:, 9 * k:9 * k + 1])
                        else:
                            nc.vector.scalar_tensor_tensor(
                                out=dst, in0=zp[0][:, 0:128],
                                scalar=usb[:, 9 * k:9 * k + 1],
                                in1=bias_rep[:],
                                op0=mybir.AluOpType.mult,
                                op1=mybir.AluOpType.add)
                    else:
                        nc.vector.scalar_tensor_tensor(
                            out=dst,
                            in0=zp[h][:, tq * 128:(tq + 1) * 128],
                            scalar=usb[:, t + 9 * k:t + 9 * k + 1],
                            in1=acc[:],
                            op0=mybir.AluOpType.mult,
                            op1=mybir.AluOpType.add)

            def store_chunk(done):
                # chunked output stores: tiles [6n, 6n+6) per DMA
                for n in range(5):
                    if done == min(6 * n + 6, NT):
                        r0 = 6 * n * 128
                        nn = done - 6 * n
                        src = outbuf[:, 6 * n * O:done * O].rearrange(
                            "p (k o) -> p k o", o=O)
                        dst = out_d[r0:r0 + nn * 128, :].rearrange(
                            "(k p) o -> p k o", p=128)
                        nc.scalar.dma_start(out=dst, in_=src)

            for k in range(NFI):
                mix_tile(k, zps.pop(k))
                store_chunk(k + 1)

            # ---- steady state: h-major conv + immediate mix ----
            for k in range(NFI, NT):
                base = PT0 + 128 * k
                zp = [ps.tile([128, 512], f32, tag="zp", bufs=7,
                              name=f"zp{h}_{k}") for h in range(2)]
                for h in range(2):
                    for ij in range(9):
                        lo = 1 + base + _delta(ij)
                        nc.tensor.matmul(
                            zp[h][:],
                            lhsT=xp[:, lo:lo + 128],
                            rhs=tbf[:, ij * 1024 + h * 512:
                                    ij * 1024 + (h + 1) * 512],
                            start=(ij == 0), stop=(ij == 8))
                mix_tile(k, zp)
                store_chunk(k + 1)

    nc.compile()
    return nc


def _get(use_alpha: int, bias_zero: int, rb_zero: int):
    key = (use_alpha, bias_zero, rb_zero)
    if key not in _cache:
        _cache[key] = _build(use_alpha, bias_zero, rb_zero)
    return _cache[key]


def _in_maps(inp):
    ua = int(np.asarray(inp["use_alpha"]))
    bz = int(not np.asarray(inp["bias"]).any())
    rz = int(not np.asarray(inp["routing_b"]).any())
    x = np.asarray(inp["inputs"], dtype=np.float32).reshape(
        NCORES, C, H, W).astype(ml_dtypes.bfloat16)
    # host-prepadded plane: image row y at pf rows 1..56, cols 0..55,
    # shifted right by 1 guard col
    xp = np.zeros((NCORES, C, PLANE), dtype=ml_dtypes.bfloat16)
    v = xp[:, :, 1:1 + NPIX].reshape(NCORES, C, HPAD, WP)
    v[:, :, 1:57, 0:W] = x
    # [O*C*3*3, T] -> [(i,j), c, t*O + o]
    tmpl = np.asarray(inp["weight_templates"], dtype=np.float32).reshape(
        O, C, 3, 3, T).transpose(1, 2, 3, 4, 0).reshape(C, 9 * T * O)
    tmpl = np.ascontiguousarray(tmpl).astype(ml_dtypes.bfloat16)
    rwt = np.ascontiguousarray(
        np.asarray(inp["routing_w"], dtype=np.float32).T)
    rb = np.ascontiguousarray(np.asarray(inp["routing_b"], dtype=np.float32))

    if ua:
        ap = np.zeros((NCORES, G, PLANE), dtype=np.float32)
        av = ap[:, :, 1:1 + NPIX].reshape(NCORES, G, HPAD, WP)
        av[:, :, 1:57, 0:W] = np.asarray(inp["Alpha"], dtype=np.float32)
    else:
        # hard routing: one-hot(mask) in plane layout; pads -> group 0
        m = np.asarray(inp["mask"]).astype(np.int64)
        ep = np.zeros((NCORES, G, PLANE), dtype=ml_dtypes.bfloat16)
        ep[:, 0, :] = 1.0
        ev = ep[:, :, 1:1 + NPIX].reshape(NCORES, G, HPAD, WP)
        oh = (m[:, None, :, :] == np.arange(G)[None, :, None, None])
        ev[:, :, 1:57, 0:W] = oh.astype(ml_dtypes.bfloat16)

    in_maps = []
    for b in range(NCORES):
        m = {"xp": xp[b], "tmpl": tmpl, "rwt": rwt}
        if not rz:
            m["rb"] = rb
        if ua:
            m["alpha"] = ap[b]
        else:
            m["eain"] = ep[b]
        if not bz:
            m["bias"] = np.ascontiguousarray(
                np.asarray(inp["bias"], dtype=np.float32))
        in_maps.append(m)
    return in_maps, ua, bz, rz


_ROWS = (np.arange(H)[:, None] * WP + np.arange(W)[None, :]).ravel()


def kernel(inputs, mask, Alpha, weight_templates, routing_w, routing_b, bias,
           use_alpha):
    in_maps, ua, bz, rz = _in_maps(dict(
        inputs=inputs, mask=mask, Alpha=Alpha,
        weight_templates=weight_templates, routing_w=routing_w,
        routing_b=routing_b, bias=bias, use_alpha=use_alpha))
    nc = _get(ua, bz, rz)
    res = run_bass_kernel_spmd(nc, in_maps, list(range(NCORES)))
    out = np.stack([res.results[b]["out"] for b in range(NCORES)], axis=0)
    # out rows are pf-57; gather valid pixels, transpose to [O, H, W]
    out = np.asarray(out, dtype=np.float32)[:, _ROWS, :]
    out = out.transpose(0, 2, 1).reshape(NCORES, O, H, W)
    return np.ascontiguousarray(out)
